# revision 1
# baseline (speedup 1.0000x reference)
"""NuFFT forward (KbNufft-style) Trainium2 Bass kernel.

Strategy:
  - Visibilities only touch |k| <= ~400 of the 2048-point oversampled grid
    (UMAX bound), so each of the 8 cores computes a 105-row x 804-col slab of
    the spectrum via DFT matmuls (apodization folded into the DFT constants):
        slab = Fv_rows . (cube/apod) . Fu_cols^T
  - Visibilities are sharded across cores by their v-row bin, so every
    core's slab fully covers its own visibilities' 6x6 KB footprints.
  - The slab is stored channel-interleaved in DRAM (row = [col][chan][re/im],
    padded to a 256B-multiple row stride); the 6x6 interpolation becomes bulk
    gpsimd.dma_gather calls (256B descriptors; visibilities binned by
    col-offset residue j0%8 so int16 indices address 64-f32-aligned starts
    from an 8*r f32 base offset), then a DVE multiply-reduce against
    host-precomputed 48-tap weight products (6 rows x 8 cols, last 2 zero).
"""
import os
import sys

for _p in ("/opt/trn_rl_repo",):
    if _p not in sys.path and os.path.isdir(_p):
        sys.path.insert(0, _p)

import numpy as np

# ---- problem constants (must match reference.py) ----
NCH = 4
NPIX = 1024
NVIS = 200_000
G = 2048
J = 6
OSF = 2
CELL_ARCSEC = 0.005
DL = CELL_ARCSEC * np.pi / (180.0 * 3600.0)
BETA = float(np.pi * np.sqrt((J / OSF) ** 2 * (OSF - 0.5) ** 2 - 0.8))

# ---- sharding geometry ----
N_CORES = 8
P = 128                      # SBUF partitions
ROW_LO_ALL = -398            # min possible m0 (floor of t), |t| < 397.2
ROWS_PER_CORE = 100
R_ROWS = ROWS_PER_CORE + 5   # 105 slab rows per core (footprint halo)
KU = 804                     # slab cols, c'_u in [-401, 403)
COL_BASE = -401
ROW_F32 = 6464               # padded slab row: 808 cols * 8 = 101*256B stride
STRIP = 408                  # stage-2 ku strip width (2 overlapping strips)
SOFF = (0, 396)              # strip col offsets; windows never straddle
N1 = 3 * R_ROWS + 1          # stage-1 rhs width (f32r needs even N)

NRES = 8                     # col-residue streams per strip
R_SLOTS = 14                 # vis slots per partition per (strip, residue)
N_STREAMS = 2 * NRES         # 16 gather streams
V_SLOTS = N_STREAMS * R_SLOTS        # 224 output rows per partition
GSTRIDE = 3328               # f32 per strip-grid row (52*256B stride)
GBLK = GSTRIDE // 64         # 52 64-f32 blocks per row
DESC_PER_S = P * R_SLOTS * J         # 10752 descriptors per stream
CALL_IDX = 1024                      # dma_gather ring capacity per call
IDXCOLS_S = DESC_PER_S // 16          # 672 int16 cols per stream
GROWS2 = (R_ROWS * GSTRIDE - 56) // 64  # 5459 64-f32 rows addressable

C1 = np.float32(1000.0 * 2.0 * np.pi * DL)   # klambda -> rad/pixel
C2 = np.float32(G / (2.0 * np.pi))           # rad/pixel -> grid coord

_NC_CACHE = {}


def _matmul_dtype():
    return os.environ.get("NUFFT_MM_DTYPE", "float32r")


def build_nc():
    """Build the SPMD Bass program (same program for all 8 cores)."""
    key = _matmul_dtype()
    if key in _NC_CACHE:
        return _NC_CACHE[key]

    import concourse.bacc as bacc
    import concourse.mybir as mybir
    import concourse.tile as tile
    from concourse import library_config
    from contextlib import ExitStack

    f32 = mybir.dt.float32
    i16 = mybir.dt.int16
    mm_dt = getattr(mybir.dt, key)

    nc = bacc.Bacc("TRN2", target_bir_lowering=False, debug=False)

    cube_d = nc.dram_tensor("cube", (NCH, NPIX, NPIX), mm_dt, kind="ExternalInput")
    cvt_d = nc.dram_tensor("cvt", (P, 8, N1), mm_dt, kind="ExternalInput")
    cut_d = nc.dram_tensor("cut", (P, 8, KU), mm_dt, kind="ExternalInput")
    sut_d = nc.dram_tensor("sut", (P, 8, KU), mm_dt, kind="ExternalInput")
    gidx_d = nc.dram_tensor("gidx", (P, N_STREAMS * IDXCOLS_S), i16,
                            kind="ExternalInput")
    w48_d = nc.dram_tensor("w48", (P, V_SLOTS, 48), f32, kind="ExternalInput")
    out_d = nc.dram_tensor("vis_out", (P, V_SLOTS, 8), f32,
                           kind="ExternalOutput")
    grid_d = [nc.dram_tensor(f"gridscratch{i}", (R_ROWS, GSTRIDE), f32)
              for i in range(2)]


    with tile.TileContext(nc) as tc:
        with ExitStack() as s12:
            # one lifetime for all pools: stage-3 tiles must NOT reuse
            # stage-1/2 SBUF zones, else their allocations pick up deps on
            # the tall/grid release (forcing gathers to wait for strip 1)
            const_pool = s12.enter_context(tc.tile_pool(name="const", bufs=1))
            cube_pool = s12.enter_context(tc.tile_pool(name="cube", bufs=3))
            tpool = s12.enter_context(tc.tile_pool(name="tmats", bufs=1))
            cpool = s12.enter_context(tc.tile_pool(name="cstream", bufs=4))
            psum_pool = s12.enter_context(
                tc.tile_pool(name="ps", bufs=8, space="PSUM"))

            cvt_sb = const_pool.tile([P, 8, N1], mm_dt)
            nc.sync.dma_start(cvt_sb[:], cvt_d[:])

            # T storage: (p, chan, term[T1,T2,negT1], xc, r)
            tall = tpool.tile([P, NCH, 3, 8, R_ROWS], mm_dt)

            # ---- stage 1: T^T = cube^T . cvt (accumulate over y chunks) ----
            for c in range(NCH):
                ps = [psum_pool.tile([P, N1], f32, tag="ps",
                                     name=f"ps1_{c}_{i}") for i in range(8)]
                for yc in range(8):
                    cb = cube_pool.tile([P, NPIX], mm_dt, tag="cube")
                    nc.sync.dma_start(cb[:], cube_d[c, yc * P:(yc + 1) * P, :])
                    for xt in range(8):
                        nc.tensor.matmul(
                            ps[xt][:],
                            lhsT=cb[:, xt * P:(xt + 1) * P],
                            rhs=cvt_sb[:, yc, :],
                            start=(yc == 0),
                            stop=(yc == 7),
                        )
                for xt in range(8):
                    for term in range(3):
                        nc.vector.tensor_copy(
                            tall[:, c, term, xt, :],
                            ps[xt][:, term * R_ROWS:(term + 1) * R_ROWS],
                        )

            # ---- stage 2: slab = T . [cut|sut], interleave, DMA to DRAM ----
            grid_sb = tpool.tile([P, KU * 8], f32)
            gv = grid_sb[:].rearrange("p (col e) -> p col e", e=8)
            zpad = cpool.tile([P, GSTRIDE - STRIP * 8], f32, tag="zpad")
            nc.gpsimd.memset(zpad[:], 0.0)
            for strip in range(2):
                off = SOFF[strip]
                ps2 = [psum_pool.tile([P, STRIP], f32, tag="ps",
                                      name=f"ps2_{strip}_{i}")
                       for i in range(8)]  # (c, re/im) -> ps2[c*2+e]
                for xc in range(8):
                    cu = cpool.tile([P, STRIP], mm_dt, tag="cu")
                    nc.sync.dma_start(
                        cu[:], cut_d[:, xc, off:off + STRIP])
                    su = cpool.tile([P, STRIP], mm_dt, tag="su")
                    nc.sync.dma_start(
                        su[:], sut_d[:, xc, off:off + STRIP])
                    for c in range(NCH):
                        t1 = tall[:, c, 0, xc, :]
                        t2 = tall[:, c, 1, xc, :]
                        nt1 = tall[:, c, 2, xc, :]
                        cuv = cu[:]
                        suv = su[:]
                        # re = T1.cu + T2.su ; im = T2.cu + (-T1).su
                        nc.tensor.matmul(ps2[c * 2][:R_ROWS, :], lhsT=t1,
                                         rhs=cuv, start=(xc == 0), stop=False)
                        nc.tensor.matmul(ps2[c * 2][:R_ROWS, :], lhsT=t2,
                                         rhs=suv, start=False, stop=(xc == 7))
                        nc.tensor.matmul(ps2[c * 2 + 1][:R_ROWS, :], lhsT=t2,
                                         rhs=cuv, start=(xc == 0), stop=False)
                        nc.tensor.matmul(ps2[c * 2 + 1][:R_ROWS, :], lhsT=nt1,
                                         rhs=suv, start=False, stop=(xc == 7))
                skip = 0 if strip == 0 else (SOFF[0] + STRIP) - SOFF[1]
                for c in range(NCH):
                    for e in range(2):
                        nc.vector.tensor_copy(
                            gv[:R_ROWS, off + skip:off + STRIP, c * 2 + e],
                            ps2[c * 2 + e][:R_ROWS, skip:],
                        )
                # ship this strip's slab so its gathers can start early
                nc.sync.dma_start(
                    grid_d[strip][:, :STRIP * 8],
                    grid_sb[:R_ROWS, off * 8:(off + STRIP) * 8])
                nc.sync.dma_start(grid_d[strip][:, STRIP * 8:],
                                  zpad[:R_ROWS, :])

            # ---- stage 3: residue-binned dma_gather + weighted reduce ----
            ipool = s12.enter_context(tc.tile_pool(name="interp", bufs=3))
            opool = s12.enter_context(tc.tile_pool(name="outp", bufs=1))

            nc.gpsimd.load_library(library_config.mlp)
            ov = opool.tile([P, V_SLOTS, 8], f32)
            flats = [grid_d[i][:, :].flatten() for i in range(2)]
            for st in range(N_STREAMS):
                sgrid, r = st // NRES, st % NRES
                view_r = flats[sgrid][8 * r: 8 * r + GROWS2 * 64].rearrange(
                    "(n e) -> n e", e=64)
                idxr = ipool.tile([P, IDXCOLS_S], i16, tag="idx",
                                  name=f"idx_{st}")
                nc.sync.dma_start(
                    idxr[:], gidx_d[:, st * IDXCOLS_S:(st + 1) * IDXCOLS_S])
                w = ipool.tile([P, R_SLOTS * 48], f32, tag="w", name=f"w_{st}")
                nc.sync.dma_start(
                    w[:],
                    w48_d[:, st * R_SLOTS:(st + 1) * R_SLOTS, :].rearrange(
                        "p v t -> p (v t)"))
                g = ipool.tile([P, R_SLOTS * J, 64], f32, tag="g",
                               name=f"g_{st}")
                done = 0
                k = 0
                while done < DESC_PER_S:
                    n_idx = min(CALL_IDX, DESC_PER_S - done)
                    nc.gpsimd.dma_gather(
                        out_ap=g[:, done // P:(done + n_idx) // P, :],
                        in_ap=view_r,
                        idxs_ap=idxr[:, done // 16:(done + n_idx) // 16],
                        num_idxs=n_idx,
                        num_idxs_reg=n_idx,
                        elem_size=64,
                        elem_step=64,
                    )
                    done += n_idx
                    k += 1
                # multiply by weights (broadcast over chan/reim)
                gw = g[:].rearrange("p t (col e) -> p (t col) e", e=8)
                wb = w[:].unsqueeze(2).to_broadcast([P, R_SLOTS * 48, 8])
                nc.vector.tensor_tensor(
                    out=gw, in0=gw, in1=wb, op=mybir.AluOpType.mult)
                # reduce over the 48 (6 rows x 8 cols, 2 zero) taps
                rv = g[:].rearrange(
                    "p (v i) (col e) -> p v e (i col)", v=R_SLOTS, i=J, e=8)
                nc.vector.tensor_reduce(
                    out=ov[:, st * R_SLOTS:(st + 1) * R_SLOTS, :],
                    in_=rv,
                    axis=mybir.AxisListType.X,
                    op=mybir.AluOpType.add,
                )
            nc.sync.dma_start(out_d[:], ov[:])

    nc.compile()
    _NC_CACHE[key] = nc
    return nc


def _apod1d():
    f = np.arange(NPIX, dtype=np.float64) / G
    z = np.pi * J * f
    s = np.sqrt(BETA * BETA - z * z)
    return J * np.sinh(s) / s  # [NPIX] float64


def _interp_host(k):
    """Match reference _interp_coords index/weight math in f32."""
    t = (k.astype(np.float32) * C1) * C2
    m0 = np.floor(t).astype(np.int32)
    offs = np.arange(J, dtype=np.int32) - (J // 2 - 1)
    d = t[:, None] - (m0[:, None] + offs).astype(np.float32)
    w = np.i0(BETA * np.sqrt(np.maximum(0.0, 1.0 - (2.0 * d / J) ** 2)))
    return t, m0, w.astype(np.float32)


def host_prep(cube, uu, vv):
    """Returns (in_maps, meta, phase) for the 8 cores."""
    mmkey = _matmul_dtype()
    if mmkey == "bfloat16":
        import ml_dtypes
        mmnp = ml_dtypes.bfloat16
    else:
        mmnp = np.float32
    cube = np.ascontiguousarray(np.asarray(cube, dtype=np.float32)).astype(mmnp)
    uu = np.asarray(uu, dtype=np.float32)
    vv = np.asarray(vv, dtype=np.float32)

    s1 = _apod1d()
    y = np.arange(NPIX, dtype=np.float64)

    # u-direction DFT constants (same for all cores)
    kj = np.arange(KU, dtype=np.float64) + COL_BASE
    ang_u = 2.0 * np.pi * np.outer(y, kj) / G
    cut = (np.cos(ang_u) / s1[:, None]).astype(np.float32)
    sut = (np.sin(ang_u) / s1[:, None]).astype(np.float32)
    cut = np.ascontiguousarray(cut.reshape(8, P, KU).transpose(1, 0, 2)).astype(mmnp)
    sut = np.ascontiguousarray(sut.reshape(8, P, KU).transpose(1, 0, 2)).astype(mmnp)

    tu, m0u, wu = _interp_host(uu)
    tv, m0v, wv = _interp_host(vv)
    assert m0u.min() >= ROW_LO_ALL and m0u.max() < ROW_LO_ALL + 8 * ROWS_PER_CORE
    assert m0v.min() >= ROW_LO_ALL and m0v.max() < ROW_LO_ALL + 8 * ROWS_PER_CORE

    core_of = (m0v - ROW_LO_ALL) // ROWS_PER_CORE
    j0 = m0u - 2 - COL_BASE        # window start col within slab, [1, 796]
    sgrid = (j0 > 400).astype(np.int64)
    colp = j0 - 396 * sgrid        # col within strip grid, [1,400] or [5,407]
    res = colp % NRES
    q = colp // NRES               # 64-f32 block within strip row, [0, 50]
    w48 = np.zeros((len(uu), J, 8), dtype=np.float32)
    w48[:, :, :J] = wv[:, :, None] * wu[:, None, :]

    in_maps = []
    meta = []
    for k in range(N_CORES):
        row_lo = ROW_LO_ALL + ROWS_PER_CORE * k
        gidx = np.zeros((P, N_STREAMS * IDXCOLS_S), dtype=np.int16)
        w48k = np.zeros((P, V_SLOTS, 48), dtype=np.float32)
        meta_k = []
        for st in range(N_STREAMS):
            sg, r = st // NRES, st % NRES
            order = np.where((core_of == k) & (sgrid == sg) & (res == r))[0]
            n = len(order)
            assert n <= P * R_SLOTS, f"core {k} stream {st} overflow: {n}"
            sl = np.arange(n)
            pp = sl % P
            vs = sl // P
            lrow = (m0v[order] - row_lo).astype(np.int64)   # [0, 100)
            vals = (lrow[:, None] + np.arange(J)[None, :]) * GBLK \
                + q[order, None].astype(np.int64)           # [n, J] <= 5458
            # descriptor t = (v*6+i)*128 + p ; idx A[t%16, t//16]
            t = (vs[:, None] * J + np.arange(J)[None, :]) * P + pp[:, None]
            block = np.zeros((16, IDXCOLS_S), dtype=np.int16)
            block[(t % 16).ravel(), (t // 16).ravel()] = vals.astype(
                np.int16).ravel()
            gidx[:, st * IDXCOLS_S:(st + 1) * IDXCOLS_S] = np.tile(block,
                                                                   (8, 1))
            w48k[pp, st * R_SLOTS + vs, :] = w48[order].reshape(n, 48)
            meta_k.append((order, pp, st * R_SLOTS + vs))
        # v-direction DFT constants for this core's rows
        kr = np.arange(R_ROWS, dtype=np.float64) + (row_lo - 2)
        ang_v = 2.0 * np.pi * np.outer(y, kr) / G
        blk = np.zeros((NPIX, 3 * R_ROWS + 1), dtype=np.float32)
        cosb = np.cos(ang_v) / s1[:, None]
        sinb = np.sin(ang_v) / s1[:, None]
        blk[:, 0 * R_ROWS:1 * R_ROWS] = cosb
        blk[:, 1 * R_ROWS:2 * R_ROWS] = -sinb
        blk[:, 2 * R_ROWS:3 * R_ROWS] = -cosb
        cvt = np.ascontiguousarray(
            blk.reshape(8, P, 3 * R_ROWS + 1).transpose(1, 0, 2)).astype(mmnp)

        in_maps.append({
            "cube": cube,
            "cvt": cvt,
            "cut": cut,
            "sut": sut,
            "gidx": gidx,
            "w48": w48k,
        })
        meta.append(meta_k)

    kv = vv * C1
    ku_ = uu * C1
    phase = np.exp(1j * (kv + ku_) * np.float32(NPIX / 2.0)).astype(np.complex64)
    return in_maps, meta, phase


def assemble(results, meta, phase):
    out = np.zeros((NCH, NVIS), dtype=np.complex64)
    for k in range(N_CORES):
        arr = results[k]["vis_out"].reshape(P, V_SLOTS, NCH, 2)
        for order, pp, rows in meta[k]:
            vals = arr[pp, rows]  # [n, NCH, 2]
            out[:, order] = (vals[..., 0] + 1j * vals[..., 1]).T
    return out * phase[None, :]


def kernel(cube, uu, vv):
    from concourse.bass_utils import run_bass_kernel_spmd

    nc = build_nc()
    in_maps, meta, phase = host_prep(cube, uu, vv)
    br = run_bass_kernel_spmd(
        nc, in_maps, list(range(N_CORES)),
        trace=bool(int(os.environ.get("NUFFT_TRACE", "0"))),
    )
    if br.exec_time_ns is not None:
        print(f"HW exec time: {br.exec_time_ns} ns")
    kernel.last_result = br
    return assemble(br.results, meta, phase)



# revision 18
# speedup vs baseline: 1.2395x; 1.2395x over previous
"""NuFFT forward (KbNufft-style) Trainium2 Bass kernel, v2.

Strategy (per core; vis sharded by |kv| row-bin after conjugate mirroring):
  - Mirror vis with m0v<0 to (-u,-v) (real image => V(-u,-v)=conj V(u,v)),
    so all vis land in m0v in [0,398). Core k owns m0v in [50k, 50k+50).
  - Stage 1 (PE, f32r): T1/T2 = cos/-sin y-transforms of cube/apod for
    this core's 55 slab rows (apod + 2^33 fp16-range scale folded into
    the DFT constants).
  - Stage 2' (PE, f32r): transposed slab sl[col_chunk 128, (row 55, ch 4,
    e 2)] via cu/su-matmuls against (T1,T2)/(T2,-T1) interleaved views;
    7 overlapping col chunks, stride 120.
  - E' expansion (PE shift-matmuls + copies): E'[c, u, ce8, i8] =
    sl[c+i, u, ce] stored fp16 in DRAM, so each vis's 6x6x(4ch x re/im)
    KB footprint is ONE contiguous 768B gather descriptor (full DMA bus
    efficiency, no <512B penalty, 6x fewer descriptors than per-row).
  - Stage 3: dma_gather (1 desc/vis, idx=(c*55+lr)//6 int16, 6 residue
    streams by (c*55+lr)%6 with 64-elem base offsets), then separable
    fp16 weighting (g*wu, reduce i; *wv, reduce u) -- separable factor
    rounding keeps the heavy KB-window cancellation from amplifying
    fp16 noise -- with f32 final accumulation.
"""
import os
import sys

for _p in ("/opt/trn_rl_repo",):
    if _p not in sys.path and os.path.isdir(_p):
        sys.path.insert(0, _p)

import numpy as np

# ---- problem constants (must match reference.py) ----
NCH = 4
NPIX = 1024
NVIS = 200_000
G = 2048
J = 6
CELL_ARCSEC = 0.005
DL = CELL_ARCSEC * np.pi / (180.0 * 3600.0)
BETA = float(np.pi * np.sqrt((J / 2) ** 2 * (2 - 0.5) ** 2 - 0.8))
C1 = np.float32(1000.0 * 2.0 * np.pi * DL)
C2 = np.float32(G / (2.0 * np.pi))

# ---- sharding / slab geometry ----
P = 128
N_CORES = 8
BIN_W = 50                 # m0v rows per core bin
SL_R = 55                  # slab rows per core (u in [0,55))
E_C = 840                  # E' col extent (7 chunks x 120)
CH_STRIDE = 120
N_CHUNK = 7
COL_OFF = 400              # col c = m0u - 2 + COL_OFF in [0, 796)
N1 = 256                   # stage-1 rhs width (3*55 terms + pad, f32r>=256)
KU2 = E_C + 8              # cu/su col extent (chunk-6 shift halo)
SUP_C = [480, 360]         # e_d rows per super-chunk group
NRES = 6
ELEM = 384                 # fp16 elems per gather descriptor (768B)
CALL_IDX = 1024            # descriptors per dma_gather call (ring cap)
WAVE = 32                  # stage-3 slots per compute wave

GS_EXP = 31                # E' scaled by 2^31 into fp16 range
WU_EXP = 16                # wu scaled by 2^-16
WV_EXP = GS_EXP - WU_EXP   # wv scaled by 2^-15

_NC_CACHE = {}


def build_nc(caps):
    """caps[s][r] = slots (128 vis each) for super s, residue stream r."""
    key = tuple(tuple(c) for c in caps)
    if key in _NC_CACHE:
        return _NC_CACHE[key]

    import concourse.bacc as bacc
    import concourse.mybir as mybir
    import concourse.tile as tile
    from concourse import library_config
    from contextlib import ExitStack

    f32 = mybir.dt.float32
    f32r = mybir.dt.float32r
    fp16 = mybir.dt.float16
    i16 = mybir.dt.int16

    TOT = sum(caps[0]) + sum(caps[1])

    nc = bacc.Bacc("TRN2", target_bir_lowering=False, debug=False)

    cube_d = nc.dram_tensor("cube", (NCH, NPIX, NPIX), f32r, kind="ExternalInput")
    cvt_d = nc.dram_tensor("cvt", (P, 8, N1), f32r, kind="ExternalInput")
    cut_d = nc.dram_tensor("cut", (P, 8, KU2), f32r, kind="ExternalInput")
    sut_d = nc.dram_tensor("sut", (P, 8, KU2), f32r, kind="ExternalInput")
    sh_d = nc.dram_tensor("sh", (P, 8, P), f32r, kind="ExternalInput")
    gidx_d = nc.dram_tensor("gidx", (P, TOT * 8), i16, kind="ExternalInput")
    wu_d = nc.dram_tensor("wu", (P, TOT, 64), fp16, kind="ExternalInput")
    wv_d = nc.dram_tensor("wv", (P, TOT, 8), fp16, kind="ExternalInput")
    out_d = nc.dram_tensor("vis_out", (P, TOT, 8), f32, kind="ExternalOutput")
    e_d = [nc.dram_tensor(f"escratch{s}", (SUP_C[s], SL_R * 64), fp16)
           for s in range(2)]

    with tile.TileContext(nc) as tc:
        with ExitStack() as st:
            const_pool = st.enter_context(tc.tile_pool(name="const", bufs=1))
            cube_pool = st.enter_context(tc.tile_pool(name="cube", bufs=2))
            tpool = st.enter_context(tc.tile_pool(name="tall", bufs=1))
            slpool = st.enter_context(tc.tile_pool(name="sl", bufs=2))
            epool = st.enter_context(tc.tile_pool(name="esb", bufs=2))
            ps_pool = st.enter_context(
                tc.tile_pool(name="ps", bufs=8, space="PSUM"))
            # stage-3 pools (kept separate; see baseline note about deps)
            gpool = st.enter_context(tc.tile_pool(name="gath", bufs=2))
            t48_pool = st.enter_context(tc.tile_pool(name="t48", bufs=2))
            opool = st.enter_context(tc.tile_pool(name="outp", bufs=1))
            wpool = st.enter_context(tc.tile_pool(name="wconst", bufs=1))

            nc.gpsimd.load_library(library_config.mlp)

            cvt_sb = const_pool.tile([P, 8, N1], f32r)
            nc.sync.dma_start(cvt_sb[:], cvt_d[:])
            cut_sb = const_pool.tile([P, 8, KU2], f32r)
            nc.sync.dma_start(cut_sb[:], cut_d[:])
            sut_sb = const_pool.tile([P, 8, KU2], f32r)
            nc.sync.dma_start(sut_sb[:], sut_d[:])
            sh_sb = const_pool.tile([P, 8, P], f32r)
            nc.sync.dma_start(sh_sb[:], sh_d[:])
            wu_sb = wpool.tile([P, TOT, 64], fp16)
            nc.sync.dma_start(wu_sb[:], wu_d[:])
            wv_sb = wpool.tile([P, TOT, 8], fp16)
            nc.sync.dma_start(wv_sb[:], wv_d[:])
            gidx_sb = wpool.tile([P, TOT * 8], i16)
            nc.sync.dma_start(gidx_sb[:], gidx_d[:])

            # ---- stage 1: T^T = cube^T . cvt (accumulate over y chunks) ----
            tall = tpool.tile([P, NCH, 3, 8, SL_R], f32r)
            for c in range(NCH):
                ps = [ps_pool.tile([P, N1], f32, tag="ps",
                                   name=f"ps1_{c}_{i}") for i in range(8)]
                for yc in range(8):
                    cb = cube_pool.tile([P, NPIX], f32r, tag="cube")
                    nc.sync.dma_start(cb[:], cube_d[c, yc * P:(yc + 1) * P, :])
                    for xt in range(8):
                        nc.tensor.matmul(
                            ps[xt][:],
                            lhsT=cb[:, xt * P:(xt + 1) * P],
                            rhs=cvt_sb[:, yc, :],
                            start=(yc == 0),
                            stop=(yc == 7),
                        )
                for xt in range(8):
                    nc.scalar.copy(
                        tall[:, c, :, xt, :],
                        ps[xt][:, 0:3 * SL_R].rearrange(
                            "p (t r) -> p t r", t=3),
                    )

            # ---- stage 2' + E' expansion, per col chunk ----
            out_sb = opool.tile([P, TOT, 8], f32)
            for ck in range(N_CHUNK):
                c0 = ck * CH_STRIDE
                ps2 = ps_pool.tile([P, SL_R * 8], f32, tag="ps",
                                   name=f"ps2_{ck}")
                for xc in range(8):
                    rhs1 = tall[:, :, 0:2, xc, :].rearrange(
                        "p c t r -> p r c t")
                    rhs2 = tall[:, :, 1:3, xc, :].rearrange(
                        "p c t r -> p r c t")
                    nc.tensor.matmul(ps2[:], lhsT=cut_sb[:, xc, c0:c0 + P],
                                     rhs=rhs1, start=(xc == 0), stop=False)
                    nc.tensor.matmul(ps2[:], lhsT=sut_sb[:, xc, c0:c0 + P],
                                     rhs=rhs2, start=False, stop=(xc == 7))
                sl_sb = slpool.tile([P, SL_R * 8], f32r, tag="sl")
                nc.scalar.copy(sl_sb[:], ps2[:])
                # E'[c, u, ce, i] = sl[c+i, u, ce] via shift matmuls
                esb = epool.tile([P, SL_R * 64], fp16, tag="esb")
                ev = esb[:].rearrange("p (u c i) -> p u c i", u=SL_R, c=8)
                for i in range(8):
                    psi = ps_pool.tile([P, SL_R * 8], f32, tag="ps",
                                       name=f"psi_{ck}_{i}")
                    nc.tensor.matmul(psi[:], lhsT=sh_sb[:, i, :], rhs=sl_sb[:],
                                     start=True, stop=True)
                    src = psi[:].rearrange("p (u c) -> p u c", u=SL_R)
                    if i % 2 == 0:
                        nc.vector.tensor_copy(ev[:, :, :, i], src)
                    else:
                        nc.scalar.copy(ev[:, :, :, i], src)
                sup = 0 if ck < 4 else 1
                r0 = c0 - (0 if sup == 0 else 480)
                nc.sync.dma_start(e_d[sup][r0:r0 + CH_STRIDE, :],
                                  esb[0:CH_STRIDE, :])

                # ---- stage 3 for a super once its last chunk is shipped ----
                if ck == 3 or ck == 6:
                    sup_done = 0 if ck == 3 else 1
                    slot0 = 0 if sup_done == 0 else sum(caps[0])
                    flat = e_d[sup_done][:, :].flatten()
                    nunit = SUP_C[sup_done] * SL_R
                    for r in range(NRES):
                        nrow = (nunit * 64 - r * 64) // ELEM
                        view_r = flat[r * 64: r * 64 + nrow * ELEM].rearrange(
                            "(n e) -> n e", e=ELEM)
                        scap = caps[sup_done][r]
                        sbase = slot0 + sum(caps[sup_done][:r])
                        w0 = 0
                        wi = 0
                        while w0 < scap:
                            nw = min(WAVE, scap - w0)
                            g = gpool.tile([P, WAVE, ELEM], fp16, tag="g",
                                           name=f"g_{sup_done}_{r}_{w0}")
                            done = 0
                            while done < nw * P:
                                n_idx = min(CALL_IDX, nw * P - done)
                                col0 = (sbase + w0) * 8 + done // 16
                                nc.gpsimd.dma_gather(
                                    out_ap=g[:, done // P:(done + n_idx) // P, :],
                                    in_ap=view_r,
                                    idxs_ap=gidx_sb[:, col0:col0 + n_idx // 16],
                                    num_idxs=n_idx,
                                    num_idxs_reg=n_idx,
                                    elem_size=ELEM,
                                    elem_step=ELEM,
                                )
                                done += n_idx
                            s0 = sbase + w0
                            # TT1: g *= wu (64-wide pattern, zeros at i=6,7),
                            # broadcast over the 6 u-units -> 3 free dims
                            gv = g[:, 0:nw, :].rearrange(
                                "p v (u x) -> p v u x", u=6)
                            wuv = wu_sb[:, s0:s0 + nw, :] \
                                .unsqueeze(2) \
                                .to_broadcast([P, nw, 6, 64])
                            tt_eng = nc.gpsimd if wi % 3 == 2 else nc.vector
                            tt_eng.tensor_tensor(
                                out=gv, in0=gv, in1=wuv,
                                op=mybir.AluOpType.mult)
                            t48 = t48_pool.tile([P, WAVE, 48], fp16, tag="t48",
                                                name=f"t_{sup_done}_{r}_{w0}")
                            t48v = t48[:, 0:nw, :].rearrange(
                                "p v (u c) -> p v u c", u=6)
                            with nc.allow_low_precision(
                                    reason="fp16 partials; verified margin"):
                                nc.vector.tensor_reduce(
                                    out=t48[:, 0:nw, :],
                                    in_=g[:, 0:nw, :].rearrange(
                                        "p v (t i) -> p v t i", i=8),
                                    axis=mybir.AxisListType.X,
                                    op=mybir.AluOpType.add)
                            wvv = wv_sb[:, s0:s0 + nw, 0:6] \
                                .unsqueeze(3) \
                                .to_broadcast([P, nw, 6, 8])
                            nc.vector.tensor_tensor(
                                out=t48v, in0=t48v, in1=wvv,
                                op=mybir.AluOpType.mult)
                            nc.vector.tensor_reduce(
                                out=out_sb[:, s0:s0 + nw, :],
                                in_=t48[:, 0:nw, :].rearrange(
                                    "p v (u c) -> p v c u", u=6),
                                axis=mybir.AxisListType.X,
                                op=mybir.AluOpType.add)
                            w0 += nw
                            wi += 1
            nc.sync.dma_start(out_d[:], out_sb[:])

    nc.compile()
    _NC_CACHE[key] = nc
    return nc


def _apod1d():
    f = np.arange(NPIX, dtype=np.float64) / G
    z = np.pi * J * f
    s = np.sqrt(BETA * BETA - z * z)
    return J * np.sinh(s) / s


def _interp_host(k):
    t = (k.astype(np.float32) * C1) * C2
    m0 = np.floor(t).astype(np.int32)
    offs = np.arange(J, dtype=np.int32) - (J // 2 - 1)
    d = t[:, None] - (m0[:, None] + offs).astype(np.float32)
    w = np.i0(BETA * np.sqrt(np.maximum(0.0, 1.0 - (2.0 * d / J) ** 2)))
    return m0, w.astype(np.float32)


def host_prep(cube, uu, vv):
    cube = np.ascontiguousarray(np.asarray(cube, dtype=np.float32))
    uu = np.asarray(uu, dtype=np.float32)
    vv = np.asarray(vv, dtype=np.float32)

    s1 = _apod1d()
    y = np.arange(NPIX, dtype=np.float64)
    gscale = float(2.0 ** GS_EXP)

    # mirror to kv>=0 half plane
    m0v0, _ = _interp_host(vv)
    mir = m0v0 < 0
    uu_e = np.where(mir, -uu, uu)
    vv_e = np.where(mir, -vv, vv)
    m0v, wv = _interp_host(vv_e)
    m0u, wu = _interp_host(uu_e)
    assert m0v.min() >= 0 and m0v.max() < N_CORES * BIN_W
    j0 = m0u - 2 + COL_OFF
    assert j0.min() >= 0 and j0.max() + 7 < E_C

    core_of = m0v // BIN_W
    lr = m0v - core_of * BIN_W
    sup_of = (j0 >= 480).astype(np.int64)

    wu_s = (wu / (2.0 ** WU_EXP)).astype(np.float16)
    wv_s = (wv / (2.0 ** WV_EXP)).astype(np.float16)

    # u-direction DFT constants (shared; carry the 2^33 fp16-range scale)
    kj = np.arange(KU2, dtype=np.float64) - COL_OFF
    ang_u = 2.0 * np.pi * np.outer(y, kj) / G
    cut = (gscale * np.cos(ang_u) / s1[:, None])
    sut = (gscale * np.sin(ang_u) / s1[:, None])
    cut = np.ascontiguousarray(
        cut.reshape(8, P, KU2).transpose(1, 0, 2)).astype(np.float32)
    sut = np.ascontiguousarray(
        sut.reshape(8, P, KU2).transpose(1, 0, 2)).astype(np.float32)

    # shift matrices: sh[i][p, f] = 1 iff p == f + i
    sh = np.zeros((P, 8, P), dtype=np.float32)
    for i in range(8):
        sh[:, i, :] = np.eye(P, P, k=-i)

    # stream caps: max count over cores per (super, res)
    U_loc = (j0 - sup_of * 480).astype(np.int64) * SL_R + lr
    res = (U_loc % NRES).astype(np.int64)
    gidx_val = (U_loc // NRES).astype(np.int64)
    counts = np.zeros((N_CORES, 2, NRES), dtype=np.int64)
    for k in range(N_CORES):
        for s in range(2):
            for r in range(NRES):
                counts[k, s, r] = np.sum(
                    (core_of == k) & (sup_of == s) & (res == r))
    caps = [[int(-(-counts[:, s, r].max() // P)) for r in range(NRES)]
            for s in range(2)]
    TOT = sum(caps[0]) + sum(caps[1])

    in_maps = []
    meta = []
    for k in range(N_CORES):
        ROW0 = BIN_W * k - 2
        kr = np.arange(SL_R, dtype=np.float64) + ROW0
        ang_v = 2.0 * np.pi * np.outer(y, kr) / G
        cosb = np.cos(ang_v) / s1[:, None]
        sinb = np.sin(ang_v) / s1[:, None]
        blk = np.zeros((NPIX, N1), dtype=np.float64)
        blk[:, 0 * SL_R:1 * SL_R] = cosb
        blk[:, 1 * SL_R:2 * SL_R] = -sinb
        blk[:, 2 * SL_R:3 * SL_R] = -cosb
        cvt = np.ascontiguousarray(
            blk.reshape(8, P, N1).transpose(1, 0, 2)).astype(np.float32)

        gidx = np.zeros((P, TOT * 8), dtype=np.int16)
        wuk = np.zeros((P, TOT, 64), dtype=np.float16)
        wvk = np.zeros((P, TOT, 8), dtype=np.float16)
        meta_k = []
        slot0 = 0
        for s in range(2):
            for r in range(NRES):
                order = np.where((core_of == k) & (sup_of == s) & (res == r))[0]
                n = len(order)
                scap = caps[s][r]
                assert n <= scap * P
                t = np.arange(n)
                pp = t % P
                ss = t // P
                vals = gidx_val[order]
                assert n == 0 or vals.max() < 32768
                block = np.zeros((16, scap * 8), dtype=np.int16)
                block[(t % 16), (t // 16)] = vals.astype(np.int16)
                gidx[:, slot0 * 8:(slot0 + scap) * 8] = np.tile(block, (8, 1))
                wu8 = np.zeros((n, 8), dtype=np.float16)
                wu8[:, :6] = wu_s[order]
                wuk[pp, slot0 + ss, :] = np.tile(wu8, (1, 8))
                wvk[pp, slot0 + ss, :6] = wv_s[order]
                meta_k.append((order, pp, slot0 + ss))
                slot0 += scap
        in_maps.append({
            "cube": cube,
            "cvt": cvt,
            "cut": cut,
            "sut": sut,
            "sh": sh,
            "gidx": gidx,
            "wu": wuk,
            "wv": wvk,
        })
        meta.append(meta_k)

    kv = vv * C1
    ku_ = uu * C1
    phase = np.exp(1j * (kv + ku_) * np.float32(NPIX / 2.0)).astype(np.complex64)
    return in_maps, meta, phase, mir, caps


def assemble(results, meta, phase, mir):
    out = np.zeros((NCH, NVIS), dtype=np.complex64)
    for k in range(N_CORES):
        arr = results[k]["vis_out"].reshape(P, -1, NCH, 2)
        for order, pp, rows in meta[k]:
            vals = arr[pp, rows]  # [n, NCH, 2]
            out[:, order] = (vals[..., 0] + 1j * vals[..., 1]).T
    out = np.where(mir[None, :], np.conj(out), out)
    return out * phase[None, :]


def kernel(cube, uu, vv):
    from concourse.bass_utils import run_bass_kernel_spmd

    in_maps, meta, phase, mir, caps = host_prep(cube, uu, vv)
    nc = build_nc(caps)
    br = run_bass_kernel_spmd(
        nc, in_maps, list(range(N_CORES)),
        trace=bool(int(os.environ.get("NUFFT_TRACE", "0"))),
    )
    if br.exec_time_ns is not None:
        print(f"HW exec time: {br.exec_time_ns} ns")
    kernel.last_result = br
    return assemble(br.results, meta, phase, mir)


# revision 31
# speedup vs baseline: 1.4506x; 1.1703x over previous
"""NuFFT forward (KbNufft-style) Trainium2 Bass kernel, v2.

Strategy (per core; vis sharded by |kv| row-bin after conjugate mirroring):
  - Mirror vis with m0v<0 to (-u,-v) (real image => V(-u,-v)=conj V(u,v)),
    so all vis land in m0v in [0,398). Core k owns m0v in [50k, 50k+50).
  - Stage 1 (PE, f32r): T1/T2 = cos/-sin y-transforms of cube/apod for
    this core's 55 slab rows (apod + 2^33 fp16-range scale folded into
    the DFT constants).
  - Stage 2' (PE, f32r): transposed slab sl[col_chunk 128, (row 55, ch 4,
    e 2)] via cu/su-matmuls against (T1,T2)/(T2,-T1) interleaved views;
    7 overlapping col chunks, stride 120.
  - E' expansion (PE shift-matmuls + copies): E'[c, u, ce8, i8] =
    sl[c+i, u, ce] stored fp16 in DRAM, so each vis's 6x6x(4ch x re/im)
    KB footprint is ONE contiguous 768B gather descriptor (full DMA bus
    efficiency, no <512B penalty, 6x fewer descriptors than per-row).
  - Stage 3: dma_gather (1 desc/vis, idx=(c*55+lr)//6 int16, 6 residue
    streams by (c*55+lr)%6 with 64-elem base offsets), then separable
    fp16 weighting (g*wu, reduce i; *wv, reduce u) -- separable factor
    rounding keeps the heavy KB-window cancellation from amplifying
    fp16 noise -- with f32 final accumulation.
"""
import os
import sys

for _p in ("/opt/trn_rl_repo",):
    if _p not in sys.path and os.path.isdir(_p):
        sys.path.insert(0, _p)

import numpy as np

# ---- problem constants (must match reference.py) ----
NCH = 4
NPIX = 1024
NVIS = 200_000
G = 2048
J = 6
CELL_ARCSEC = 0.005
DL = CELL_ARCSEC * np.pi / (180.0 * 3600.0)
BETA = float(np.pi * np.sqrt((J / 2) ** 2 * (2 - 0.5) ** 2 - 0.8))
C1 = np.float32(1000.0 * 2.0 * np.pi * DL)
C2 = np.float32(G / (2.0 * np.pi))

# ---- sharding / slab geometry ----
P = 128
N_CORES = 8
BIN_W = 50                 # m0v rows per core bin
SL_R = 55                  # slab rows per core (u in [0,55))
E_C = 840                  # E' col extent (7 chunks x 120)
CH_STRIDE = 120
N_CHUNK = 7
COL_OFF = 400              # col c = m0u - 2 + COL_OFF in [0, 796)
N1 = 256                   # stage-1 rhs width (3*55 terms + pad, f32r>=256)
KU2 = E_C + 8              # cu/su col extent (chunk-6 shift halo)
SUP_C = [480, 360]         # e_d rows per super-chunk group
NRES = 8                   # residue streams: (c*55+lr) % 8
UNIT = 48                  # fp16 elems per (c,u) unit: [ce 8, i 6]
ELEM = 384                 # fp16 elems per gather descriptor (768B, 8 units)
CALL_IDX = 1024            # descriptors per dma_gather call (ring cap)
WAVE = 32                  # stage-3 slots per compute wave

GS_EXP = 31                # E' scaled by 2^31 into fp16 range
WU_EXP = 16                # wu scaled by 2^-16
WV_EXP = GS_EXP - WU_EXP   # wv scaled by 2^-15

_NC_CACHE = {}


def build_nc(caps):
    """caps[s][r] = slots (128 vis each) for super s, residue stream r."""
    key = tuple(tuple(c) for c in caps)
    if key in _NC_CACHE:
        return _NC_CACHE[key]

    import concourse.bacc as bacc
    import concourse.mybir as mybir
    import concourse.tile as tile
    from concourse import library_config
    from contextlib import ExitStack

    f32 = mybir.dt.float32
    f32r = mybir.dt.float32r
    fp16 = mybir.dt.float16
    i16 = mybir.dt.int16

    TOT = sum(caps[0]) + sum(caps[1])

    nc = bacc.Bacc("TRN2", target_bir_lowering=False, debug=False)

    cube_d = nc.dram_tensor("cube", (NCH, NPIX, NPIX), f32r, kind="ExternalInput")
    cvt_d = nc.dram_tensor("cvt", (P, 8, N1), f32r, kind="ExternalInput")
    cut_d = nc.dram_tensor("cut", (P, 8, KU2), f32r, kind="ExternalInput")
    sut_d = nc.dram_tensor("sut", (P, 8, KU2), f32r, kind="ExternalInput")
    sh_d = nc.dram_tensor("sh", (P, 8, P), f32r, kind="ExternalInput")
    gidx_d = nc.dram_tensor("gidx", (P, TOT * 8), i16, kind="ExternalInput")
    wu_d = nc.dram_tensor("wu", (P, TOT, UNIT), fp16, kind="ExternalInput")
    wv_d = nc.dram_tensor("wv", (P, TOT, 8), fp16, kind="ExternalInput")
    out_d = nc.dram_tensor("vis_out", (P, TOT, 8), f32, kind="ExternalOutput")
    # +1 guard row: the 768B descriptor of a vis in the last col reads 2
    # junk units past its 6 real ones (never consumed by TT/TR)
    e_d = [nc.dram_tensor(f"escratch{s}", (SUP_C[s] + 1, SL_R * UNIT), fp16)
           for s in range(2)]

    with tile.TileContext(nc) as tc:
        with ExitStack() as st:
            const_pool = st.enter_context(tc.tile_pool(name="const", bufs=1))
            cube_pool = st.enter_context(tc.tile_pool(name="cube", bufs=2))
            tpool = st.enter_context(tc.tile_pool(name="tall", bufs=1))
            slpool = st.enter_context(tc.tile_pool(name="sl", bufs=2))
            epool = st.enter_context(tc.tile_pool(name="esb", bufs=2))
            ps_pool = st.enter_context(
                tc.tile_pool(name="ps", bufs=8, space="PSUM"))
            # stage-3 pools (kept separate; see baseline note about deps)
            gpool = st.enter_context(tc.tile_pool(name="gath", bufs=2))
            t48_pool = st.enter_context(tc.tile_pool(name="t48", bufs=2))
            opool = st.enter_context(tc.tile_pool(name="outp", bufs=1))
            wpool = st.enter_context(tc.tile_pool(name="wconst", bufs=1))

            nc.gpsimd.load_library(library_config.mlp)

            cvt_sb = const_pool.tile([P, 8, N1], f32r)
            nc.sync.dma_start(cvt_sb[:], cvt_d[:])
            cut_sb = const_pool.tile([P, 8, KU2], f32r)
            sut_sb = const_pool.tile([P, 8, KU2], f32r)
            sh_sb = const_pool.tile([P, 8, P], f32r)
            wu_sb = wpool.tile([P, TOT, UNIT], fp16)
            wv_sb = wpool.tile([P, TOT, 8], fp16)
            gidx_sb = wpool.tile([P, TOT * 8], i16)

            # ---- stage 1: T^T = cube^T . cvt (accumulate over y chunks) ----
            # const loads interleaved between channels so the first cube
            # chunks aren't queued behind ~13MB of stage-2/3 constants
            tall = tpool.tile([P, NCH, 3, 8, SL_R], f32r)
            for c in range(NCH):
                ps = [ps_pool.tile([P, N1], f32, tag="ps",
                                   name=f"ps1_{c}_{i}") for i in range(8)]
                for yc in range(8):
                    cb = cube_pool.tile([P, NPIX], f32r, tag="cube")
                    nc.sync.dma_start(cb[:], cube_d[c, yc * P:(yc + 1) * P, :])
                    for xt in range(8):
                        nc.tensor.matmul(
                            ps[xt][:],
                            lhsT=cb[:, xt * P:(xt + 1) * P],
                            rhs=cvt_sb[:, yc, :],
                            start=(yc == 0),
                            stop=(yc == 7),
                        )
                if c == 0:
                    nc.sync.dma_start(cut_sb[:], cut_d[:])
                elif c == 1:
                    nc.sync.dma_start(sut_sb[:], sut_d[:])
                elif c == 2:
                    nc.sync.dma_start(sh_sb[:], sh_d[:])
                    nc.sync.dma_start(gidx_sb[:], gidx_d[:])
                else:
                    nc.sync.dma_start(wu_sb[:], wu_d[:])
                    nc.sync.dma_start(wv_sb[:], wv_d[:])
                for xt in range(8):
                    nc.scalar.copy(
                        tall[:, c, :, xt, :],
                        ps[xt][:, 0:3 * SL_R].rearrange(
                            "p (t r) -> p t r", t=3),
                    )

            # ---- stage 2' + E' expansion, per col chunk ----
            out_sb = opool.tile([P, TOT, 8], f32)
            for ck in range(N_CHUNK):
                c0 = ck * CH_STRIDE
                ps2 = ps_pool.tile([P, SL_R * 8], f32, tag="ps",
                                   name=f"ps2_{ck}")
                for xc in range(8):
                    rhs1 = tall[:, :, 0:2, xc, :].rearrange(
                        "p c t r -> p r c t")
                    rhs2 = tall[:, :, 1:3, xc, :].rearrange(
                        "p c t r -> p r c t")
                    nc.tensor.matmul(ps2[:], lhsT=cut_sb[:, xc, c0:c0 + P],
                                     rhs=rhs1, start=(xc == 0), stop=False)
                    nc.tensor.matmul(ps2[:], lhsT=sut_sb[:, xc, c0:c0 + P],
                                     rhs=rhs2, start=False, stop=(xc == 7))
                sl_sb = slpool.tile([P, SL_R * 8], f32r, tag="sl")
                nc.scalar.copy(sl_sb[:], ps2[:])
                # E'[c, u, ce, i] = sl[c+i, u, ce] via shift matmuls
                esb = epool.tile([P, SL_R * UNIT], fp16, tag="esb")
                ev = esb[:].rearrange("p (u c i) -> p u c i", u=SL_R, c=8)
                for i in range(J):
                    psi = ps_pool.tile([P, SL_R * 8], f32, tag="ps",
                                       name=f"psi_{ck}_{i}")
                    nc.tensor.matmul(psi[:], lhsT=sh_sb[:, i, :], rhs=sl_sb[:],
                                     start=True, stop=True)
                    src = psi[:].rearrange("p (u c) -> p u c", u=SL_R)
                    if i % 2 == 0:
                        nc.vector.tensor_copy(ev[:, :, :, i], src)
                    else:
                        nc.scalar.copy(ev[:, :, :, i], src)
                sup = 0 if ck < 4 else 1
                r0 = c0 - (0 if sup == 0 else 480)
                # final chunk of each super also fills the +1 guard row
                # (tail-overrun descriptor bytes; never consumed by TT/TR)
                nrows = CH_STRIDE + (1 if ck in (3, 6) else 0)
                nc.sync.dma_start(e_d[sup][r0:r0 + nrows, :],
                                  esb[0:nrows, :])

                # ---- stage 3 for a super once its last chunk is shipped ----
                if ck == 3 or ck == 6:
                    sup_done = 0 if ck == 3 else 1
                    slot0 = 0 if sup_done == 0 else sum(caps[0])
                    flat = e_d[sup_done][:, :].flatten()
                    nunit = (SUP_C[sup_done] + 1) * SL_R
                    for r in range(NRES):
                        nrow = (nunit * UNIT - r * UNIT) // ELEM
                        view_r = flat[r * UNIT: r * UNIT + nrow * ELEM] \
                            .rearrange("(n e) -> n e", e=ELEM)
                        scap = caps[sup_done][r]
                        sbase = slot0 + sum(caps[sup_done][:r])
                        w0 = 0
                        wi = 0
                        while w0 < scap:
                            nw = min(WAVE, scap - w0)
                            g = gpool.tile([P, WAVE, ELEM], fp16, tag="g",
                                           name=f"g_{sup_done}_{r}_{w0}")
                            done = 0
                            while done < nw * P:
                                n_idx = min(CALL_IDX, nw * P - done)
                                col0 = (sbase + w0) * 8 + done // 16
                                nc.gpsimd.dma_gather(
                                    out_ap=g[:, done // P:(done + n_idx) // P, :],
                                    in_ap=view_r,
                                    idxs_ap=gidx_sb[:, col0:col0 + n_idx // 16],
                                    num_idxs=n_idx,
                                    num_idxs_reg=n_idx,
                                    elem_size=ELEM,
                                    elem_step=ELEM,
                                )
                                done += n_idx
                            s0 = sbase + w0
                            # TT1: g *= wu ([ce 8, i 6] pattern), broadcast
                            # over the 6 u-units -> 3 free dims
                            g288 = g[:, 0:nw, 0:6 * UNIT]
                            gv = g288.rearrange("p v (u x) -> p v u x", u=6)
                            wuv = wu_sb[:, s0:s0 + nw, :] \
                                .unsqueeze(2) \
                                .to_broadcast([P, nw, 6, UNIT])
                            tt_eng = nc.gpsimd if wi % 3 == 2 else nc.vector
                            tt_eng.tensor_tensor(
                                out=gv, in0=gv, in1=wuv,
                                op=mybir.AluOpType.mult)
                            t48 = t48_pool.tile([P, WAVE, 48], fp16, tag="t48",
                                                name=f"t_{sup_done}_{r}_{w0}")
                            t48v = t48[:, 0:nw, :].rearrange(
                                "p v (u c) -> p v u c", u=6)
                            with nc.allow_low_precision(
                                    reason="fp16 partials; verified margin"):
                                nc.vector.tensor_reduce(
                                    out=t48[:, 0:nw, :],
                                    in_=g288.rearrange(
                                        "p v (t i) -> p v t i", i=6),
                                    axis=mybir.AxisListType.X,
                                    op=mybir.AluOpType.add)
                            wvv = wv_sb[:, s0:s0 + nw, 0:6] \
                                .unsqueeze(3) \
                                .to_broadcast([P, nw, 6, 8])
                            t48f = t48_pool.tile([P, WAVE, 48], f32,
                                                 tag="t48f",
                                                 name=f"tf_{sup_done}_{r}_{w0}")
                            nc.vector.tensor_tensor(
                                out=t48f[:, 0:nw, :].rearrange(
                                    "p v (u c) -> p v u c", u=6),
                                in0=t48v, in1=wvv,
                                op=mybir.AluOpType.mult)
                            nc.vector.tensor_reduce(
                                out=out_sb[:, s0:s0 + nw, :],
                                in_=t48f[:, 0:nw, :].rearrange(
                                    "p v (u c) -> p v c u", u=6),
                                axis=mybir.AxisListType.X,
                                op=mybir.AluOpType.add)
                            w0 += nw
                            wi += 1
            nc.sync.dma_start(out_d[:], out_sb[:])

    nc.compile()
    _NC_CACHE[key] = nc
    return nc


def _apod1d():
    f = np.arange(NPIX, dtype=np.float64) / G
    z = np.pi * J * f
    s = np.sqrt(BETA * BETA - z * z)
    return J * np.sinh(s) / s


def _interp_host(k):
    t = (k.astype(np.float32) * C1) * C2
    m0 = np.floor(t).astype(np.int32)
    offs = np.arange(J, dtype=np.int32) - (J // 2 - 1)
    d = t[:, None] - (m0[:, None] + offs).astype(np.float32)
    w = np.i0(BETA * np.sqrt(np.maximum(0.0, 1.0 - (2.0 * d / J) ** 2)))
    return m0, w.astype(np.float32)


def host_prep(cube, uu, vv):
    cube = np.ascontiguousarray(np.asarray(cube, dtype=np.float32))
    uu = np.asarray(uu, dtype=np.float32)
    vv = np.asarray(vv, dtype=np.float32)

    s1 = _apod1d()
    y = np.arange(NPIX, dtype=np.float64)
    gscale = float(2.0 ** GS_EXP)

    # mirror to kv>=0 half plane
    m0v0, _ = _interp_host(vv)
    mir = m0v0 < 0
    uu_e = np.where(mir, -uu, uu)
    vv_e = np.where(mir, -vv, vv)
    m0v, wv = _interp_host(vv_e)
    m0u, wu = _interp_host(uu_e)
    assert m0v.min() >= 0 and m0v.max() < N_CORES * BIN_W
    j0 = m0u - 2 + COL_OFF
    assert j0.min() >= 0 and j0.max() + 7 < E_C

    core_of = m0v // BIN_W
    lr = m0v - core_of * BIN_W
    sup_of = (j0 >= 480).astype(np.int64)

    wu_s = (wu / (2.0 ** WU_EXP)).astype(np.float16)
    wv_s = (wv / (2.0 ** WV_EXP)).astype(np.float16)

    # u-direction DFT constants (shared; carry the 2^33 fp16-range scale)
    kj = np.arange(KU2, dtype=np.float64) - COL_OFF
    ang_u = 2.0 * np.pi * np.outer(y, kj) / G
    cut = (gscale * np.cos(ang_u) / s1[:, None])
    sut = (gscale * np.sin(ang_u) / s1[:, None])
    cut = np.ascontiguousarray(
        cut.reshape(8, P, KU2).transpose(1, 0, 2)).astype(np.float32)
    sut = np.ascontiguousarray(
        sut.reshape(8, P, KU2).transpose(1, 0, 2)).astype(np.float32)

    # shift matrices: sh[i][p, f] = 1 iff p == f + i
    sh = np.zeros((P, 8, P), dtype=np.float32)
    for i in range(8):
        sh[:, i, :] = np.eye(P, P, k=-i)

    # stream caps: max count over cores per (super, res)
    U_loc = (j0 - sup_of * 480).astype(np.int64) * SL_R + lr
    res = (U_loc % NRES).astype(np.int64)
    gidx_val = (U_loc // NRES).astype(np.int64)
    counts = np.zeros((N_CORES, 2, NRES), dtype=np.int64)
    for k in range(N_CORES):
        for s in range(2):
            for r in range(NRES):
                counts[k, s, r] = np.sum(
                    (core_of == k) & (sup_of == s) & (res == r))
    caps = [[int(-(-counts[:, s, r].max() // P)) for r in range(NRES)]
            for s in range(2)]
    TOT = sum(caps[0]) + sum(caps[1])

    in_maps = []
    meta = []
    for k in range(N_CORES):
        ROW0 = BIN_W * k - 2
        kr = np.arange(SL_R, dtype=np.float64) + ROW0
        ang_v = 2.0 * np.pi * np.outer(y, kr) / G
        cosb = np.cos(ang_v) / s1[:, None]
        sinb = np.sin(ang_v) / s1[:, None]
        blk = np.zeros((NPIX, N1), dtype=np.float64)
        blk[:, 0 * SL_R:1 * SL_R] = cosb
        blk[:, 1 * SL_R:2 * SL_R] = -sinb
        blk[:, 2 * SL_R:3 * SL_R] = -cosb
        cvt = np.ascontiguousarray(
            blk.reshape(8, P, N1).transpose(1, 0, 2)).astype(np.float32)

        gidx = np.zeros((P, TOT * 8), dtype=np.int16)
        wuk = np.zeros((P, TOT, UNIT), dtype=np.float16)
        wvk = np.zeros((P, TOT, 8), dtype=np.float16)
        meta_k = []
        slot0 = 0
        for s in range(2):
            for r in range(NRES):
                order = np.where((core_of == k) & (sup_of == s) & (res == r))[0]
                n = len(order)
                scap = caps[s][r]
                assert n <= scap * P
                t = np.arange(n)
                pp = t % P
                ss = t // P
                vals = gidx_val[order]
                assert n == 0 or vals.max() < 32768
                block = np.zeros((16, scap * 8), dtype=np.int16)
                block[(t % 16), (t // 16)] = vals.astype(np.int16)
                gidx[:, slot0 * 8:(slot0 + scap) * 8] = np.tile(block, (8, 1))
                wuk[pp, slot0 + ss, :] = np.tile(wu_s[order], (1, 8))
                wvk[pp, slot0 + ss, :6] = wv_s[order]
                meta_k.append((order, pp, slot0 + ss))
                slot0 += scap
        in_maps.append({
            "cube": cube,
            "cvt": cvt,
            "cut": cut,
            "sut": sut,
            "sh": sh,
            "gidx": gidx,
            "wu": wuk,
            "wv": wvk,
        })
        meta.append(meta_k)

    kv = vv * C1
    ku_ = uu * C1
    phase = np.exp(1j * (kv + ku_) * np.float32(NPIX / 2.0)).astype(np.complex64)
    return in_maps, meta, phase, mir, caps


def assemble(results, meta, phase, mir):
    out = np.zeros((NCH, NVIS), dtype=np.complex64)
    for k in range(N_CORES):
        arr = results[k]["vis_out"].reshape(P, -1, NCH, 2)
        for order, pp, rows in meta[k]:
            vals = arr[pp, rows]  # [n, NCH, 2]
            out[:, order] = (vals[..., 0] + 1j * vals[..., 1]).T
    out = np.where(mir[None, :], np.conj(out), out)
    return out * phase[None, :]


def kernel(cube, uu, vv):
    from concourse.bass_utils import run_bass_kernel_spmd

    in_maps, meta, phase, mir, caps = host_prep(cube, uu, vv)
    nc = build_nc(caps)
    br = run_bass_kernel_spmd(
        nc, in_maps, list(range(N_CORES)),
        trace=bool(int(os.environ.get("NUFFT_TRACE", "0"))),
    )
    if br.exec_time_ns is not None:
        print(f"HW exec time: {br.exec_time_ns} ns")
    kernel.last_result = br
    return assemble(br.results, meta, phase, mir)
